# revision 10
# baseline (speedup 1.0000x reference)
"""Locally-connected graph-conv kernel for Trainium2 (Bass/Tile).

Computes out[b,t,m] = sum_n x[b,t,n] * (S*W)[n,m] + bias[m] for
x [64, 2048, 208], W/S [208, 208], bias [208].

The ring-graph support S is a +-4 band (mod 208), so each half of the
output nodes only needs a 112-row slice of the contraction dim. With a
rotated node layout (row j holds node (j-4) mod 208, 216 rows total):
  block 0 (m 0..103):   rotated rows   0..111
  block 1 (m 104..207): rotated rows 104..215
Each output block is a SINGLE [112,104] x [112,512] matmul with the
host-premasked weight block stationary in the PE array and x^T streaming
as the moving operand.

Everything that touches HBM is bf16 (PSUM accumulation stays fp32):
the 2e-2 rel-err budget dwarfs bf16 rounding (~5e-3), and it halves DMA
bytes vs fp32. HBM per NeuronCore is ~358 GB/s (and ramps up over the
first ~20 us), so the ~14.9 MB/core of traffic floors the kernel at
~45 us; everything else is shaped to stay under that:
 - all x loads are issued up-front into persistent SBUF tiles (x fits:
   2 x 32 KB/partition) on the Sync ring, so the load stream runs at
   whatever rate HBM gives with zero dependency stalls; graduated chunk
   sizes (2 KB cols first, 4 KB mid) start compute early and keep the
   pipeline tail short.
 - weights/bias DRAM rows are padded to 2 KB so their one-time loads
   are not tiny-descriptor crawls that clog a ring (wh first on Scalar,
   bias on GpSimd).
 - PSUM->SBUF eviction is stuck at 1 elem/lane/cycle (fp32 PSUM source),
   so block 0 evicts on VectorE and block 1 on ScalarE. Output blocks
   are stored non-overlapping ([224, SHARD]): block 0 on the Sync ring
   (queued after all loads), block 1 on the Scalar ring, so neither
   evicting engine ever blocks on the other's semaphore.
 - 4 dummy matmuls on the weight tile right after it lands warm the PE
   HAM (cold 1.2 GHz -> warm 2.4 GHz) before real data arrives.
The host transposes y^T back at gather.
"""

import numpy as np
import ml_dtypes
from contextlib import ExitStack

import concourse.bacc as bacc
import concourse.mybir as mybir
import concourse.tile as tile
from concourse.bass_utils import run_bass_kernel_spmd

N = 208                      # nodes
HALF = 104                   # output nodes per block
K = 4                        # band half-width of S
NH = 2 * K + HALF            # 112 contraction rows per block (halo incl.)
NR = N + 2 * K               # 216 rotated rows
WPAD = 1024                  # wh DRAM row padding (2 KB rows -> fast DMA)
BPAD = 512                   # bias DRAM row padding (2 KB f32 rows)
N_CORES = 8
B, T = 64, 2048
ROWS_TOTAL = B * T           # 131072
SHARD = ROWS_TOTAL // N_CORES    # 16384 rows per core
TB = 512                     # moving-block columns per matmul (fp32 PSUM max)
TB2 = 2 * TB                 # eviction group (2 PSUM banks)
CHUNKS = [2048, 2048, 4096, 4096, 2048, 2048]   # t-cols per pipeline chunk
assert sum(CHUNKS) == SHARD
N_DUMMY = 4                  # PE warm-up matmuls on the weight tile

FP32 = mybir.dt.float32
BF16 = mybir.dt.bfloat16
NP_BF16 = ml_dtypes.bfloat16
IDENT = mybir.ActivationFunctionType.Identity

# halo row order (indices into the [208] node dim) for each block
ROWS0 = list(range(N - K, N)) + list(range(0, HALF + K))          # 112
ROWS1 = list(range(HALF - K, N)) + list(range(0, K))              # 112

_CACHE = {}
LAST_RESULTS = None          # BassKernelResults of the most recent run


def _kernel_body(tc):
    nc = tc.nc
    # rotated x: row j = node (j-4) mod 208; block0 = rows 0:112,
    # block1 = rows 104:216
    x_d = nc.dram_tensor("xh", [NR, SHARD], BF16, kind="ExternalInput").ap()
    w_d = nc.dram_tensor("wh", [NH, WPAD], BF16, kind="ExternalInput").ap()
    b_d = nc.dram_tensor("bias", [2 * NH, BPAD], FP32, kind="ExternalInput").ap()
    o_d = nc.dram_tensor("outt", [2 * NH, SHARD], BF16, kind="ExternalOutput").ap()

    with ExitStack() as ctx:
        const = ctx.enter_context(tc.tile_pool(name="const", bufs=1))

        # One-time setup, all padded to >=2KB DMA descriptors: weights
        # first on the Scalar ring; bias halves ride the Sync ring right
        # after the first chunk (the GpSimd SWDGE path signals completion
        # ~7us late and gated the whole ScalarE eviction stream).
        wh = const.tile([NH, WPAD], BF16, tag="wh")
        nc.scalar.dma_start(wh, w_d)
        bA = const.tile([NH, BPAD], FP32, tag="bA")
        bB = const.tile([NH, BPAD], FP32, tag="bB")
        bAc = bA[0:HALF, 0:1]
        bBc = bB[0:HALF, 0:1]

        ps0p = ctx.enter_context(tc.tile_pool(name="ps0p", bufs=4, space="PSUM"))
        ps1p = ctx.enter_context(tc.tile_pool(name="ps1p", bufs=4, space="PSUM"))

        # All x loads up-front into persistent tiles, block0 on the Sync
        # ring and block1 on the Scalar ring. Output tiles are persistent
        # too (everything fits in SBUF), so no eviction ever waits on a
        # store and no store ever waits on a tile-reuse WAR.
        xts = []
        col = 0
        for c, csz in enumerate(CHUNKS):
            lsl = slice(col, col + csz)
            xh0 = const.tile([NH, csz], BF16, tag=f"xh0_{c}")
            xh1 = const.tile([NH, csz], BF16, tag=f"xh1_{c}")
            nc.sync.dma_start(xh0, x_d[0:NH, lsl])
            nc.scalar.dma_start(xh1, x_d[HALF:NR, lsl])
            if c == 0:
                nc.sync.dma_start(bA, b_d[0:NH, :])
                nc.sync.dma_start(bB, b_d[NH : 2 * NH, :])
            xts.append((xh0, xh1, col, csz))
            col += csz

        # PE warm-up: HAM un-throttles (1.2 -> 2.4 GHz) after ~3.4us of
        # sustained busy; burn idle pre-data time on the weight tile.
        for _ in range(N_DUMMY):
            psd = ps0p.tile([HALF, TB], FP32, tag="ps0")
            nc.tensor.matmul(psd, wh[:, 0:HALF], wh[:, 0:TB], start=True, stop=True)

        for c, (xh0, xh1, col, csz) in enumerate(xts):
            tsl = slice(col, col + csz)
            o0_t = const.tile([NH, csz], BF16, tag=f"o0_{c}")
            o1_t = const.tile([NH, csz], BF16, tag=f"o1_{c}")
            for s in range(csz // TB):
                g = slice(s * TB, (s + 1) * TB)
                # [104, 512] PSUM tiles (1 bank each, 4 in flight/block)
                ps0 = ps0p.tile([HALF, TB], FP32, tag="ps0")
                nc.tensor.matmul(ps0, wh[:, 0:HALF], xh0[:, g], start=True, stop=True)
                ps1 = ps1p.tile([HALF, TB], FP32, tag="ps1")
                nc.tensor.matmul(ps1, wh[:, HALF:N], xh1[:, g], start=True, stop=True)
                # evictions split across engines; both fuse bias + fp32->bf16
                nc.vector.tensor_scalar_add(o0_t[0:HALF, g], ps0, bAc)
                nc.scalar.activation(o1_t[0:HALF, g], ps1, IDENT, bias=bBc)
            # non-overlapping stores on separate rings: block0 on Sync
            # (rides behind the loads), block1 on Scalar
            nc.sync.dma_start(o_d[0:NH, tsl], o0_t)
            nc.scalar.dma_start(o_d[NH : 2 * NH, tsl], o1_t)


def _build():
    nc = bacc.Bacc(
        "TRN2",
        target_bir_lowering=False,
        debug=False,
        num_devices=N_CORES,
    )
    with tile.TileContext(nc) as tc:
        _kernel_body(tc)
    nc.compile()
    return nc


def kernel(x, W, b, S):
    global LAST_RESULTS
    nc = _CACHE.get("nc")
    if nc is None:
        nc = _build()
        _CACHE["nc"] = nc

    xf = np.asarray(x, np.float32).reshape(ROWS_TOTAL, N)
    SW = (np.asarray(S, np.float32) * np.asarray(W, np.float32))
    wh = np.zeros((NH, WPAD), NP_BF16)
    wh[:, 0:HALF] = SW[ROWS0, 0:HALF]
    wh[:, HALF:N] = SW[ROWS1, HALF:N]
    bfv = np.asarray(b, np.float32).reshape(N)
    bf = np.zeros((2 * NH, BPAD), np.float32)
    bf[0:HALF, 0] = bfv[0:HALF]
    bf[NH : NH + HALF, 0] = bfv[HALF:N]

    in_maps = []
    for i in range(N_CORES):
        xt = xf[i * SHARD : (i + 1) * SHARD].T          # [208, SHARD] view
        xh = np.empty((NR, SHARD), NP_BF16)
        xh[0:K] = xt[N - K : N]
        xh[K : N + K] = xt
        xh[N + K : NR] = xt[0:K]
        in_maps.append({"xh": xh, "wh": wh, "bias": bf})
    res = run_bass_kernel_spmd(nc, in_maps, core_ids=list(range(N_CORES)))
    LAST_RESULTS = res
    out = np.empty((ROWS_TOTAL, N), np.float32)
    for i, r in enumerate(res.results):
        yt = r["outt"]                                  # [224, SHARD] bf16
        out[i * SHARD : (i + 1) * SHARD, 0:HALF] = yt[0:HALF].T
        out[i * SHARD : (i + 1) * SHARD, HALF:N] = yt[NH : NH + HALF].T
    return out.reshape(B, T, N)


# revision 15
# speedup vs baseline: 1.0704x; 1.0704x over previous
"""Locally-connected graph-conv kernel for Trainium2 (Bass/Tile).

Computes out[b,t,m] = sum_n x[b,t,n] * (S*W)[n,m] + bias[m] for
x [64, 2048, 208], W/S [208, 208], bias [208].

The ring-graph support S is a +-4 band (mod 208), so each half of the
output nodes only needs a 112-row slice of the contraction dim. With a
rotated node layout (row j holds node (j-4) mod 208, 216 rows total):
  block 0 (m 0..103):   rotated rows   0..111
  block 1 (m 104..207): rotated rows 104..215
Each output block is a SINGLE [112,104] x [112,512] matmul with the
host-premasked weight block stationary in the PE array and x^T streaming
as the moving operand.

Everything that touches HBM is bf16 (PSUM accumulation stays fp32):
the 2e-2 rel-err budget dwarfs bf16 rounding (~5e-3), and it halves DMA
bytes vs fp32. HBM per NeuronCore is ~358 GB/s (and ramps up over the
first ~20 us), so the ~14.9 MB/core of traffic floors the kernel at
~45 us; everything else is shaped to stay under that:
 - all x loads are issued up-front into persistent SBUF tiles (x fits:
   2 x 32 KB/partition) on the Sync ring, so the load stream runs at
   whatever rate HBM gives with zero dependency stalls; graduated chunk
   sizes (2 KB cols first, 4 KB mid) start compute early and keep the
   pipeline tail short.
 - weights/bias DRAM rows are padded to 2 KB so their one-time loads
   are not tiny-descriptor crawls that clog a ring (wh first on Scalar,
   bias on GpSimd).
 - PSUM->SBUF eviction is stuck at 1 elem/lane/cycle (fp32 PSUM source),
   so block 0 evicts on VectorE and block 1 on ScalarE. Output blocks
   are stored non-overlapping ([224, SHARD]): block 0 on the Sync ring
   (queued after all loads), block 1 on the Scalar ring, so neither
   evicting engine ever blocks on the other's semaphore.
 - 4 dummy matmuls on the weight tile right after it lands warm the PE
   HAM (cold 1.2 GHz -> warm 2.4 GHz) before real data arrives.
The host transposes y^T back at gather.
"""

import numpy as np
import ml_dtypes
from contextlib import ExitStack

import concourse.bacc as bacc
import concourse.mybir as mybir
import concourse.tile as tile
from concourse.bass_utils import run_bass_kernel_spmd

N = 208                      # nodes
HALF = 104                   # output nodes per block
K = 4                        # band half-width of S
NH = 2 * K + HALF            # 112 contraction rows per block (halo incl.)
NR = N + 2 * K               # 216 rotated rows
WPAD = 1024                  # wh DRAM row padding (2 KB rows -> fast DMA)
BPAD = 256                   # bias DRAM row padding (1 KB f32 rows)
N_CORES = 8
B, T = 64, 2048
ROWS_TOTAL = B * T           # 131072
SHARD = ROWS_TOTAL // N_CORES    # 16384 rows per core
TB = 512                     # moving-block columns per matmul (fp32 PSUM max)
TB2 = 2 * TB                 # eviction group (2 PSUM banks)
CHUNKS = [2048, 2048, 4096, 4096, 2048, 2048]   # t-cols per pipeline chunk
assert sum(CHUNKS) == SHARD
N_DUMMY = 8                  # PE warm-up matmuls on the weight tile

FP32 = mybir.dt.float32
BF16 = mybir.dt.bfloat16
NP_BF16 = ml_dtypes.bfloat16
IDENT = mybir.ActivationFunctionType.Identity

# halo row order (indices into the [208] node dim) for each block
ROWS0 = list(range(N - K, N)) + list(range(0, HALF + K))          # 112
ROWS1 = list(range(HALF - K, N)) + list(range(0, K))              # 112

_CACHE = {}
LAST_RESULTS = None          # BassKernelResults of the most recent run


def _kernel_body(tc):
    nc = tc.nc
    # rotated x: row j = node (j-4) mod 208; block0 = rows 0:112,
    # block1 = rows 104:216
    x_d = nc.dram_tensor("xh", [NR, SHARD], BF16, kind="ExternalInput").ap()
    w_d = nc.dram_tensor("wh", [NH, WPAD], BF16, kind="ExternalInput").ap()
    b_d = nc.dram_tensor("bias", [2 * NH, BPAD], FP32, kind="ExternalInput").ap()
    o_d = nc.dram_tensor("outt", [2 * NH, SHARD], BF16, kind="ExternalOutput").ap()

    with ExitStack() as ctx:
        const = ctx.enter_context(tc.tile_pool(name="const", bufs=1))

        # One-time setup, all padded to >=2KB DMA descriptors: weights
        # first on the Scalar ring; bias halves ride the Sync ring right
        # after the first chunk (the GpSimd SWDGE path signals completion
        # ~7us late and gated the whole ScalarE eviction stream).
        wh = const.tile([NH, WPAD], BF16, tag="wh")
        nc.scalar.dma_start(wh, w_d)
        bA = const.tile([NH, BPAD], FP32, tag="bA")
        bB = const.tile([NH, BPAD], FP32, tag="bB")
        bAc = bA[0:HALF, 0:1]
        bBc = bB[0:HALF, 0:1]

        ps0p = ctx.enter_context(tc.tile_pool(name="ps0p", bufs=2, space="PSUM"))
        ps1p = ctx.enter_context(tc.tile_pool(name="ps1p", bufs=2, space="PSUM"))

        # All x loads up-front into persistent tiles, block0 on the Sync
        # ring and block1 on the Scalar ring. Output tiles are persistent
        # too (everything fits in SBUF), so no eviction ever waits on a
        # store and no store ever waits on a tile-reuse WAR.
        xts = []
        col = 0
        for c, csz in enumerate(CHUNKS):
            lsl = slice(col, col + csz)
            xh0 = const.tile([NH, csz], BF16, tag=f"xh0_{c}")
            xh1 = const.tile([NH, csz], BF16, tag=f"xh1_{c}")
            nc.sync.dma_start(xh0, x_d[0:NH, lsl])
            nc.scalar.dma_start(xh1, x_d[HALF:NR, lsl])
            if c == 0:
                nc.sync.dma_start(bA, b_d[0:NH, :])
                nc.scalar.dma_start(bB, b_d[NH : 2 * NH, :])
            xts.append((xh0, xh1, col, csz))
            col += csz

        # PE warm-up: HAM un-throttles (1.2 -> 2.4 GHz) after ~3.4us of
        # sustained busy; burn idle pre-data time on the weight tile.
        for _ in range(N_DUMMY):
            psd = ps0p.tile([HALF, TB2], FP32, tag="ps0")
            nc.tensor.matmul(psd[:, 0:TB], wh[:, 0:HALF], wh[:, 0:TB], start=True, stop=True)

        for c, (xh0, xh1, col, csz) in enumerate(xts):
            tsl = slice(col, col + csz)
            o0_t = const.tile([NH, csz], BF16, tag=f"o0_{c}")
            o1_t = const.tile([NH, csz], BF16, tag=f"o1_{c}")
            for s in range(csz // TB2):
                g = slice(s * TB2, (s + 1) * TB2)
                ga = slice(s * TB2, s * TB2 + TB)
                gb = slice(s * TB2 + TB, (s + 1) * TB2)
                # [104, 1024] PSUM tiles (2 banks); one matmul per bank
                ps0 = ps0p.tile([HALF, TB2], FP32, tag="ps0")
                nc.tensor.matmul(ps0[:, 0:TB], wh[:, 0:HALF], xh0[:, ga], start=True, stop=True)
                nc.tensor.matmul(ps0[:, TB:TB2], wh[:, 0:HALF], xh0[:, gb], start=True, stop=True)
                ps1 = ps1p.tile([HALF, TB2], FP32, tag="ps1")
                nc.tensor.matmul(ps1[:, 0:TB], wh[:, HALF:N], xh1[:, ga], start=True, stop=True)
                nc.tensor.matmul(ps1[:, TB:TB2], wh[:, HALF:N], xh1[:, gb], start=True, stop=True)
                # evictions split across engines; both fuse bias + fp32->bf16
                nc.vector.tensor_scalar_add(o0_t[0:HALF, g], ps0, bAc)
                nc.scalar.activation(o1_t[0:HALF, g], ps1, IDENT, bias=bBc)
            # non-overlapping stores on separate rings: block0 on Sync
            # (rides behind the loads), block1 on Scalar
            nc.sync.dma_start(o_d[0:NH, tsl], o0_t)
            nc.scalar.dma_start(o_d[NH : 2 * NH, tsl], o1_t)


def _build():
    nc = bacc.Bacc(
        "TRN2",
        target_bir_lowering=False,
        debug=False,
        num_devices=N_CORES,
    )
    with tile.TileContext(nc) as tc:
        _kernel_body(tc)
    nc.compile()
    return nc


def kernel(x, W, b, S):
    global LAST_RESULTS
    nc = _CACHE.get("nc")
    if nc is None:
        nc = _build()
        _CACHE["nc"] = nc

    xf = np.asarray(x, np.float32).reshape(ROWS_TOTAL, N)
    SW = (np.asarray(S, np.float32) * np.asarray(W, np.float32))
    wh = np.zeros((NH, WPAD), NP_BF16)
    wh[:, 0:HALF] = SW[ROWS0, 0:HALF]
    wh[:, HALF:N] = SW[ROWS1, HALF:N]
    bfv = np.asarray(b, np.float32).reshape(N)
    bf = np.zeros((2 * NH, BPAD), np.float32)
    bf[0:HALF, 0] = bfv[0:HALF]
    bf[NH : NH + HALF, 0] = bfv[HALF:N]

    in_maps = []
    for i in range(N_CORES):
        xt = xf[i * SHARD : (i + 1) * SHARD].T          # [208, SHARD] view
        xh = np.empty((NR, SHARD), NP_BF16)
        xh[0:K] = xt[N - K : N]
        xh[K : N + K] = xt
        xh[N + K : NR] = xt[0:K]
        in_maps.append({"xh": xh, "wh": wh, "bias": bf})
    res = run_bass_kernel_spmd(nc, in_maps, core_ids=list(range(N_CORES)))
    LAST_RESULTS = res
    out = np.empty((ROWS_TOTAL, N), np.float32)
    for i, r in enumerate(res.results):
        yt = r["outt"]                                  # [224, SHARD] bf16
        out[i * SHARD : (i + 1) * SHARD, 0:HALF] = yt[0:HALF].T
        out[i * SHARD : (i + 1) * SHARD, HALF:N] = yt[NH : NH + HALF].T
    return out.reshape(B, T, N)


# revision 17
# speedup vs baseline: 1.1138x; 1.0406x over previous
"""Locally-connected graph-conv kernel for Trainium2 (Bass/Tile).

Computes out[b,t,m] = sum_n x[b,t,n] * (S*W)[n,m] + bias[m] for
x [64, 2048, 208], W/S [208, 208], bias [208].

The ring-graph support S is a +-4 band (mod 208), so each half of the
output nodes only needs a 112-row slice of the contraction dim. With a
rotated node layout (row j holds node (j-4) mod 208, 216 rows total):
  block 0 (m 0..103):   rotated rows   0..111
  block 1 (m 104..207): rotated rows 104..215
Each output block is a SINGLE [112,104] x [112,512] matmul with the
host-premasked weight block stationary in the PE array and x^T streaming
as the moving operand.

Everything that touches HBM is bf16 (PSUM accumulation stays fp32):
the 2e-2 rel-err budget dwarfs bf16 rounding (~5e-3), and it halves DMA
bytes vs fp32. HBM per NeuronCore is ~358 GB/s (and ramps up over the
first ~20 us), so the ~14.9 MB/core of traffic floors the kernel at
~45 us; everything else is shaped to stay under that:
 - all x loads are issued up-front into persistent SBUF tiles (x fits:
   2 x 32 KB/partition) on the Sync ring, so the load stream runs at
   whatever rate HBM gives with zero dependency stalls; graduated chunk
   sizes (2 KB cols first, 4 KB mid) start compute early and keep the
   pipeline tail short.
 - weights/bias DRAM rows are padded to 2 KB so their one-time loads
   are not tiny-descriptor crawls that clog a ring (wh first on Scalar,
   bias on GpSimd).
 - PSUM->SBUF eviction is stuck at 1 elem/lane/cycle (fp32 PSUM source),
   so block 0 evicts on VectorE and block 1 on ScalarE. Output blocks
   are stored non-overlapping ([224, SHARD]): block 0 on the Sync ring
   (queued after all loads), block 1 on the Scalar ring, so neither
   evicting engine ever blocks on the other's semaphore.
 - 4 dummy matmuls on the weight tile right after it lands warm the PE
   HAM (cold 1.2 GHz -> warm 2.4 GHz) before real data arrives.
The host transposes y^T back at gather.
"""

import numpy as np
import ml_dtypes
from contextlib import ExitStack

import concourse.bacc as bacc
import concourse.mybir as mybir
import concourse.tile as tile
from concourse.bass_utils import run_bass_kernel_spmd

N = 208                      # nodes
HALF = 104                   # output nodes per block
K = 4                        # band half-width of S
NH = 2 * K + HALF            # 112 contraction rows per block (halo incl.)
NR = N + 2 * K               # 216 rotated rows
WPAD = 1024                  # wh DRAM row padding (2 KB rows -> fast DMA)
BPAD = 256                   # bias DRAM row padding (1 KB f32 rows)
N_CORES = 8
B, T = 64, 2048
ROWS_TOTAL = B * T           # 131072
SHARD = ROWS_TOTAL // N_CORES    # 16384 rows per core
TB = 512                     # moving-block columns per matmul (fp32 PSUM max)
TB2 = 2 * TB                 # eviction group (2 PSUM banks)
CHUNKS = [2048, 2048, 4096, 4096, 2048, 2048]   # t-cols per pipeline chunk
assert sum(CHUNKS) == SHARD
N_DUMMY = 8                  # PE warm-up matmuls on the weight tile

FP32 = mybir.dt.float32
BF16 = mybir.dt.bfloat16
NP_BF16 = ml_dtypes.bfloat16
IDENT = mybir.ActivationFunctionType.Identity

# halo row order (indices into the [208] node dim) for each block
ROWS0 = list(range(N - K, N)) + list(range(0, HALF + K))          # 112
ROWS1 = list(range(HALF - K, N)) + list(range(0, K))              # 112

_CACHE = {}
LAST_RESULTS = None          # BassKernelResults of the most recent run


def _kernel_body(tc):
    nc = tc.nc
    # rotated x: row j = node (j-4) mod 208; block0 = rows 0:112,
    # block1 = rows 104:216
    x_d = nc.dram_tensor("xh", [NR, SHARD], BF16, kind="ExternalInput").ap()
    w_d = nc.dram_tensor("wh", [NH, WPAD], BF16, kind="ExternalInput").ap()
    b_d = nc.dram_tensor("bias", [2 * NH, BPAD], FP32, kind="ExternalInput").ap()
    o_d = nc.dram_tensor("outt", [2 * NH, SHARD], BF16, kind="ExternalOutput").ap()

    with ExitStack() as ctx:
        const = ctx.enter_context(tc.tile_pool(name="const", bufs=1))

        # One-time setup, all padded to >=1KB DMA descriptors, all on the
        # Scalar ring which carries no loads: weights first (dummies need
        # them), then the bias halves -- they delay no x load and land
        # well before the first eviction. (GpSimd SWDGE signals completion
        # ~7us late, so it gets nothing latency-sensitive.)
        wh = const.tile([NH, WPAD], BF16, tag="wh")
        nc.scalar.dma_start(wh, w_d)
        bA = const.tile([NH, BPAD], FP32, tag="bA")
        bB = const.tile([NH, BPAD], FP32, tag="bB")
        nc.scalar.dma_start(bA, b_d[0:NH, :])
        nc.scalar.dma_start(bB, b_d[NH : 2 * NH, :])
        bAc = bA[0:HALF, 0:1]
        bBc = bB[0:HALF, 0:1]

        ps0p = ctx.enter_context(tc.tile_pool(name="ps0p", bufs=2, space="PSUM"))
        ps1p = ctx.enter_context(tc.tile_pool(name="ps1p", bufs=2, space="PSUM"))

        # All x loads up-front on the Sync ring (the ring carries nothing
        # else, so the stream runs at whatever HBM gives), in natural
        # consumption order, into persistent tiles. Output tiles are
        # persistent too (everything fits in SBUF), so no eviction ever
        # waits on a store and no store ever waits on a tile-reuse WAR.
        xts = []
        col = 0
        for c, csz in enumerate(CHUNKS):
            lsl = slice(col, col + csz)
            xh0 = const.tile([NH, csz], BF16, tag=f"xh0_{c}")
            xh1 = const.tile([NH, csz], BF16, tag=f"xh1_{c}")
            nc.sync.dma_start(xh0, x_d[0:NH, lsl])
            nc.sync.dma_start(xh1, x_d[HALF:NR, lsl])
            xts.append((xh0, xh1, col, csz))
            col += csz

        # PE warm-up: HAM un-throttles (1.2 -> 2.4 GHz) after ~3.4us of
        # sustained busy; burn idle pre-data time on the weight tile.
        for _ in range(N_DUMMY):
            psd = ps0p.tile([HALF, TB2], FP32, tag="ps0")
            nc.tensor.matmul(psd[:, 0:TB], wh[:, 0:HALF], wh[:, 0:TB], start=True, stop=True)

        for c, (xh0, xh1, col, csz) in enumerate(xts):
            tsl = slice(col, col + csz)
            o0_t = const.tile([NH, csz], BF16, tag=f"o0_{c}")
            o1_t = const.tile([NH, csz], BF16, tag=f"o1_{c}")
            for s in range(csz // TB2):
                g = slice(s * TB2, (s + 1) * TB2)
                ga = slice(s * TB2, s * TB2 + TB)
                gb = slice(s * TB2 + TB, (s + 1) * TB2)
                # [104, 1024] PSUM tiles (2 banks); one matmul per bank
                ps0 = ps0p.tile([HALF, TB2], FP32, tag="ps0")
                nc.tensor.matmul(ps0[:, 0:TB], wh[:, 0:HALF], xh0[:, ga], start=True, stop=True)
                nc.tensor.matmul(ps0[:, TB:TB2], wh[:, 0:HALF], xh0[:, gb], start=True, stop=True)
                ps1 = ps1p.tile([HALF, TB2], FP32, tag="ps1")
                nc.tensor.matmul(ps1[:, 0:TB], wh[:, HALF:N], xh1[:, ga], start=True, stop=True)
                nc.tensor.matmul(ps1[:, TB:TB2], wh[:, HALF:N], xh1[:, gb], start=True, stop=True)
                # evictions split across engines; both fuse bias + fp32->bf16
                nc.vector.tensor_scalar_add(o0_t[0:HALF, g], ps0, bAc)
                nc.scalar.activation(o1_t[0:HALF, g], ps1, IDENT, bias=bBc)
            # all stores on the Scalar ring, block1 first (its evictions
            # ran on ScalarE itself, so the issue never cross-engine
            # stalls; block0's VectorE evictions are long done by then)
            nc.scalar.dma_start(o_d[NH : 2 * NH, tsl], o1_t)
            nc.scalar.dma_start(o_d[0:NH, tsl], o0_t)


def _build():
    nc = bacc.Bacc(
        "TRN2",
        target_bir_lowering=False,
        debug=False,
        num_devices=N_CORES,
    )
    with tile.TileContext(nc) as tc:
        _kernel_body(tc)
    nc.compile()
    return nc


def kernel(x, W, b, S):
    global LAST_RESULTS
    nc = _CACHE.get("nc")
    if nc is None:
        nc = _build()
        _CACHE["nc"] = nc

    xf = np.asarray(x, np.float32).reshape(ROWS_TOTAL, N)
    SW = (np.asarray(S, np.float32) * np.asarray(W, np.float32))
    wh = np.zeros((NH, WPAD), NP_BF16)
    wh[:, 0:HALF] = SW[ROWS0, 0:HALF]
    wh[:, HALF:N] = SW[ROWS1, HALF:N]
    bfv = np.asarray(b, np.float32).reshape(N)
    bf = np.zeros((2 * NH, BPAD), np.float32)
    bf[0:HALF, 0] = bfv[0:HALF]
    bf[NH : NH + HALF, 0] = bfv[HALF:N]

    in_maps = []
    for i in range(N_CORES):
        xt = xf[i * SHARD : (i + 1) * SHARD].T          # [208, SHARD] view
        xh = np.empty((NR, SHARD), NP_BF16)
        xh[0:K] = xt[N - K : N]
        xh[K : N + K] = xt
        xh[N + K : NR] = xt[0:K]
        in_maps.append({"xh": xh, "wh": wh, "bias": bf})
    res = run_bass_kernel_spmd(nc, in_maps, core_ids=list(range(N_CORES)))
    LAST_RESULTS = res
    out = np.empty((ROWS_TOTAL, N), np.float32)
    for i, r in enumerate(res.results):
        yt = r["outt"]                                  # [224, SHARD] bf16
        out[i * SHARD : (i + 1) * SHARD, 0:HALF] = yt[0:HALF].T
        out[i * SHARD : (i + 1) * SHARD, HALF:N] = yt[NH : NH + HALF].T
    return out.reshape(B, T, N)
